# revision 20
# baseline (speedup 1.0000x reference)
"""Trainium2 Bass kernel for causal multi-head attention with ALiBi.

Module: y = proj(softmax((causal_mask(q k^T) + alibi)/sqrt(dh)) v) + b
Shapes: x [2, 2048, 1024], H=16 heads, dh=64, fp32.

Sharding over 8 cores: core c -> batch b = c//4, head group g = c%4
(heads 4g..4g+3). Each core computes QKV for its 4 heads, causal ALiBi
attention, and its row-slice of the output projection; the host sums the
4 row-parallel partials per batch (b_proj is added by the g==0 cores).

Device-side structure:
- All matmuls float32r (full PE rate; plain fp32 runs at 1/4).
- q/k head-transposed [65, 2048]: rows 0:64 head dims, row 64 is an
  augmentation row (k: ones, q: -slope*i) so the scores matmul yields
  qk - slope*i; the per-column term cancels in softmax and only bounds
  the exponents. The +slope*j ALiBi half enters exactly through the ACT
  per-partition bias. Scores are computed transposed [j, i].
- exp with no max pass; causal handled by skipping fully-masked j-tiles
  and one wide affine_select zero-fill over the 4 diagonal j-tiles per
  (i-slice, head), which are exp'd into one contiguous [128, 2048] tile.
- ctx matmul lhsT = [64 v-cols | 64 ones-cols]: psum rows 0:64 get the
  unnormalized context, rows 64:128 get 64 replicated copies of the
  softmax denominator -> reciprocal + one DVE multiply normalizes.
- Phase A/B fused per 512-row group: x DMA -> PE transpose -> q/k matmul
  for that i-slice -> v matmul for those j-tiles, so DMA and compute
  overlap from the start.
- Phase C: two heads' streams interleave, and each block's tail work
  (diagonal ctx, reciprocal, normalize, projection) drains into the next
  block's scores stream so the PE never stalls on the mask/normalize
  chain.
"""

import numpy as np

import concourse.bass as bass
import concourse.mybir as mybir
import concourse.tile as tile
from concourse import bacc
from concourse.bass_utils import run_bass_kernel_spmd

F32 = mybir.dt.float32
F32R = mybir.dt.float32r
AF = mybir.ActivationFunctionType

B, T, D = 2, 2048, 1024
H = 16
DH = 64
HL = 4            # heads per core
NCORES = 8
NT = T // 128     # 16 j-tiles
NS = T // 512     # 4 i-slices
KC = D // 128     # 8 contraction chunks over D
CTX_LAG = 3       # ctx matmuls trail their exp by this many emission steps
DRAIN_PER_STEP = 2


def build_nc():
    nc = bacc.Bacc("TRN2", target_bir_lowering=False, debug=False)

    x_d = nc.dram_tensor("x", [T, D], F32R, kind="ExternalInput")
    wq_d = nc.dram_tensor("wq", [D, HL * DH], F32R, kind="ExternalInput")
    wk_d = nc.dram_tensor("wk", [D, HL * DH], F32R, kind="ExternalInput")
    wv_d = nc.dram_tensor("wv", [D, HL * DH], F32R, kind="ExternalInput")
    wp_d = nc.dram_tensor("wp", [HL * DH, D], F32R, kind="ExternalInput")
    qaug_d = nc.dram_tensor("qaug", [HL, T], F32R, kind="ExternalInput")
    kones_d = nc.dram_tensor("kones", [1, T], F32R, kind="ExternalInput")
    vones_d = nc.dram_tensor("vones", [128, 64], F32R, kind="ExternalInput")
    iden_d = nc.dram_tensor("iden", [128, 128], F32R, kind="ExternalInput")
    bj_d = nc.dram_tensor("bj", [HL, 128, NT], F32, kind="ExternalInput")
    bias_d = nc.dram_tensor("bias", [128, D], F32, kind="ExternalInput")
    out_d = nc.dram_tensor("out", [T, D], F32, kind="ExternalOutput")

    with tile.TileContext(nc) as tc:
        with tc.tile_pool(name="persist", bufs=1) as pp:
            qTa = [pp.tile([65, T], F32R, name=f"qTa{h}") for h in range(HL)]
            kTa = [pp.tile([65, T], F32R, name=f"kTa{h}") for h in range(HL)]
            vA = [pp.tile([128, 384], F32R, name=f"vA{j}") for j in range(NT)]

            hwq = [nc.sync, nc.scalar]

            def load_constants():
                for h in range(HL):
                    nc.gpsimd.dma_start(qTa[h][64:65, :], qaug_d[h : h + 1, :])
                    nc.gpsimd.dma_start(kTa[h][64:65, :], kones_d[:, :])
                for j in range(NT):
                    nc.gpsimd.dma_start(vA[j][:, 64:128], vones_d[:])
                    nc.gpsimd.dma_start(vA[j][:, 256:320], vones_d[:])

            # ---- Phase A+B fused: per row-group transpose + QKV ----
            with (
                tc.tile_pool(name="ab", bufs=1) as ab,
                tc.tile_pool(name="abr", bufs=7) as abr,
                tc.tile_pool(name="psab", bufs=2, space="PSUM") as psab,
            ):
                idn = ab.tile([128, 128], F32R, name="idn")
                nc.scalar.dma_start(idn[:], iden_d[:])
                xT = [ab.tile([128, T], F32R, name=f"xT{d}") for d in range(KC)]
                wqs = [ab.tile([128, HL * DH], F32R, name=f"wqs{d}") for d in range(KC)]
                wks = [ab.tile([128, HL * DH], F32R, name=f"wks{d}") for d in range(KC)]
                wvs = [ab.tile([128, HL * DH], F32R, name=f"wvs{d}") for d in range(KC)]

                def load_weights():
                    n = 0
                    for d in range(KC):
                        hwq[n % 2].dma_start(
                            wqs[d][:], wq_d[d * 128 : (d + 1) * 128, :]
                        )
                        hwq[(n + 1) % 2].dma_start(
                            wks[d][:], wk_d[d * 128 : (d + 1) * 128, :]
                        )
                        hwq[n % 2].dma_start(
                            wvs[d][:], wv_d[d * 128 : (d + 1) * 128, :]
                        )
                        n += 1

                for sg in range(NS):
                    # 1. DMA this row-group's x tiles (alternating HW queues)
                    xts = []
                    for i4 in range(4):
                        it = sg * 4 + i4
                        xt = abr.tile([128, D], F32R, name="xt", tag="xt")
                        hwq[i4 % 2].dma_start(
                            xt[:], x_d[it * 128 : (it + 1) * 128, :]
                        )
                        xts.append(xt)
                    if sg == 0:
                        load_weights()
                        load_constants()

                    # 2. transpose into xT[:, sg-slice] via PE matmul with I
                    for d in range(KC):
                        pt = psab.tile([128, 512], F32, name="pt", tag="pt")
                        for i4 in range(4):
                            nc.tensor.matmul(
                                pt[:, i4 * 128 : (i4 + 1) * 128],
                                xts[i4][:, d * 128 : (d + 1) * 128],
                                idn[:],
                                start=True,
                                stop=True,
                            )
                        nc.vector.tensor_copy(
                            xT[d][:, sg * 512 : (sg + 1) * 512], pt[:]
                        )

                    # 3. q/k projections for i-slice sg
                    for ws, dst in ((wqs, qTa), (wks, kTa)):
                        for p2 in range(2):
                            pq = psab.tile([128, 512], F32, name="pq", tag="pq")
                            for d in range(KC):
                                nc.tensor.matmul(
                                    pq[:],
                                    ws[d][:, p2 * 128 : (p2 + 1) * 128],
                                    xT[d][:, sg * 512 : (sg + 1) * 512],
                                    start=(d == 0),
                                    stop=(d == KC - 1),
                                )
                            nc.vector.tensor_copy(
                                dst[2 * p2][0:64, sg * 512 : (sg + 1) * 512],
                                pq[0:64, :],
                            )
                            nc.vector.tensor_copy(
                                dst[2 * p2 + 1][0:64, sg * 512 : (sg + 1) * 512],
                                pq[64:128, :],
                            )

                    # 4. v projection for this group's j-tiles
                    for i4 in range(4):
                        j = sg * 4 + i4
                        pv = psab.tile([128, HL * DH], F32, name="pv", tag="pv")
                        for d in range(KC):
                            nc.tensor.matmul(
                                pv[:],
                                xT[d][:, j * 128 : (j + 1) * 128],
                                wvs[d][:],
                                start=(d == 0),
                                stop=(d == KC - 1),
                            )
                        nc.vector.tensor_copy(vA[j][:, 0:64], pv[:, 0:64])
                        nc.vector.tensor_copy(vA[j][:, 128:256], pv[:, 64:192])
                        nc.vector.tensor_copy(vA[j][:, 320:384], pv[:, 192:256])

            # ---- Phase C+D: attention + projection, cross-block pipelined ----
            with (
                tc.tile_pool(name="attn", bufs=1) as at,
                tc.tile_pool(name="attnr", bufs=4) as atr,
                tc.tile_pool(name="pssc", bufs=3, space="PSUM") as pssc,
                tc.tile_pool(name="pscx", bufs=3, space="PSUM") as pscx,
                tc.tile_pool(name="psot", bufs=2, space="PSUM") as psot,
            ):
                wps = [at.tile([128, D], F32R, name=f"wps{t}") for t in range(2)]
                bias_sb = at.tile([128, D], F32, name="bias_sb")
                ctxT = [at.tile([128, T], F32R, name=f"ctxT{t}") for t in range(2)]
                bjs = [at.tile([128, NT], F32, name=f"bjs{h}") for h in range(HL)]
                for t in range(2):
                    nc.sync.dma_start(wps[t][:], wp_d[t * 128 : (t + 1) * 128, :])
                nc.sync.dma_start(bias_sb[:], bias_d[:])
                for h in range(HL):
                    nc.sync.dma_start(bjs[h][:], bj_d[h])

                pending = []  # deferred tail closures, drained into next block
                proj_queue = []  # proj units wait one extra block

                def drain(k):
                    for _ in range(k):
                        if pending:
                            pending.pop(0)()

                def emit_head_pair(s, heads, promote_projs=False):
                    """Interleave two heads' scores/exp/ctx; defer the diag-ctx
                    + normalize tail into the next block via `pending`."""
                    if promote_projs:
                        pending.extend(proj_queue)
                        proj_queue.clear()
                    nj = 4 * (s + 1)
                    diag = list(range(4 * s, 4 * s + 4))
                    off = list(range(0, 4 * s))
                    jseq = off + diag          # scores/exp emission order
                    cons = off + diag          # ctx emission order
                    st = {
                        h: dict(
                            pcx=pscx.tile([128, 512], F32, name="pcx", tag="pcx"),
                            exd=atr.tile(
                                [128, 2048], F32R, name="exd", tag="exd", bufs=4
                            ),
                            exs={},
                            exstep={},
                            ci=0,
                        )
                        for h in heads
                    }

                    def scores_exp(h, idx, step):
                        j = jseq[idx]
                        i = st[h]
                        psc = pssc.tile(
                            [128, 512], F32, name="psc", tag="psc", bufs=3
                        )
                        nc.tensor.matmul(
                            psc[:],
                            kTa[h][:, j * 128 : (j + 1) * 128],
                            qTa[h][:, s * 512 : (s + 1) * 512],
                            start=True,
                            stop=True,
                        )
                        if idx >= nj - 4:
                            dst = i["exd"][:, (idx - (nj - 4)) * 512 : (idx - (nj - 4) + 1) * 512]
                        else:
                            dst = atr.tile(
                                [128, 512], F32R, name="ex", tag="ex", bufs=6
                            )[:]
                        nc.scalar.activation(
                            dst, psc[:], AF.Exp,
                            bias=bjs[h][:, j : j + 1], scale=0.125,
                        )
                        i["exs"][j] = dst
                        i["exstep"][j] = step
                        if idx == nj - 1:
                            # one mask over all 4 diagonal tiles:
                            # keep where f - 128*blk - p >= 0
                            nc.gpsimd.affine_select(
                                out=i["exd"][:], in_=i["exd"][:],
                                compare_op=mybir.AluOpType.is_ge,
                                fill=0.0, base=0,
                                pattern=[[-128, 4], [1, 512]],
                                channel_multiplier=-1,
                            )

                    VA_OFF = [0, 64, 192, 256]

                    def ctx_one(h, ci):
                        i = st[h]
                        j = cons[ci]
                        o = VA_OFF[h]
                        nc.tensor.matmul(
                            i["pcx"][:],
                            vA[j][:, o : o + 128],
                            i["exs"][j][:],
                            start=(ci == 0),
                            stop=(ci == nj - 1),
                        )

                    def try_offdiag_ctx(step_now):
                        for h in heads:
                            i = st[h]
                            while i["ci"] < len(off):  # off-diag only
                                j = cons[i["ci"]]
                                if j not in i["exs"]:
                                    break
                                if step_now < i["exstep"][j] + CTX_LAG:
                                    break
                                ctx_one(h, i["ci"])
                                i["ci"] += 1

                    step = 0
                    for idx in range(nj):
                        for h in heads:
                            scores_exp(h, idx, step)
                            step += 1
                            drain(DRAIN_PER_STEP)
                            try_offdiag_ctx(step)

                    # tail: remaining off-diag + diag ctx, then normalize
                    def make_tail(h):
                        i = st[h]

                        def fin_ctx(ci):
                            return lambda: ctx_one(h, ci)

                        items = [fin_ctx(ci) for ci in range(i["ci"], nj)]
                        i["ci"] = nj

                        def norm():
                            rs = atr.tile([64, 512], F32, name="rs", tag="rs")
                            ctx_rows = (0, 64) if h % 2 == 0 else (64, 128)
                            rs_rows = (64, 128) if h % 2 == 0 else (0, 64)
                            nc.vector.reciprocal(
                                rs[:], i["pcx"][rs_rows[0] : rs_rows[1], :]
                            )
                            nc.vector.tensor_mul(
                                ctxT[h // 2][
                                    (h % 2) * 64 : (h % 2) * 64 + 64,
                                    s * 512 : (s + 1) * 512,
                                ],
                                i["pcx"][ctx_rows[0] : ctx_rows[1], :],
                                rs[:],
                            )

                        items.append(norm)
                        return items

                    for h in heads:
                        pending.extend(make_tail(h))

                def proj_unit(s, i4, n2):
                    def run():
                        it = s * 4 + i4
                        po = psot.tile([128, 512], F32, name="po", tag="po")
                        for t in range(2):
                            nc.tensor.matmul(
                                po[:],
                                ctxT[t][:, it * 128 : (it + 1) * 128],
                                wps[t][:, n2 * 512 : (n2 + 1) * 512],
                                start=(t == 0),
                                stop=(t == 1),
                            )
                        ot = atr.tile([128, 512], F32, name="ot", tag="ot")
                        nc.vector.tensor_add(
                            ot[:], po[:], bias_sb[:, n2 * 512 : (n2 + 1) * 512]
                        )
                        nc.sync.dma_start(
                            out_d[
                                it * 128 : (it + 1) * 128,
                                n2 * 512 : (n2 + 1) * 512,
                            ],
                            ot[:],
                        )

                    return run

                for s in range(NS):
                    emit_head_pair(s, (0, 1))
                    emit_head_pair(s, (2, 3), promote_projs=True)
                    for i4 in range(4):
                        for n2 in range(2):
                            proj_queue.append(proj_unit(s, i4, n2))
                drain(len(pending))
                pending.extend(proj_queue)
                proj_queue.clear()
                drain(len(pending))
    nc.compile()
    return nc


_NC = None


def get_nc():
    global _NC
    if _NC is None:
        _NC = build_nc()
    return _NC


def make_in_maps(x, W_query, W_key, W_value, W_proj, b_proj):
    x = np.asarray(x, dtype=np.float32)
    W_query = np.asarray(W_query, dtype=np.float32)
    W_key = np.asarray(W_key, dtype=np.float32)
    W_value = np.asarray(W_value, dtype=np.float32)
    W_proj = np.asarray(W_proj, dtype=np.float32)
    b_proj = np.asarray(b_proj, dtype=np.float32)

    slopes = 2.0 ** (-8.0 * np.arange(1, H + 1, dtype=np.float32) / H)
    i_idx = np.arange(T, dtype=np.float32)
    p_idx = np.arange(128, dtype=np.float32)
    jt_idx = np.arange(NT, dtype=np.float32)
    iden = np.eye(128, dtype=np.float32)
    vones = np.ones((128, 64), dtype=np.float32)
    kones = np.ones((1, T), dtype=np.float32)

    in_maps = []
    for c in range(NCORES):
        b, g = divmod(c, HL)
        hs = slopes[g * HL : (g + 1) * HL]  # [4]
        qaug = -hs[:, None] * i_idx[None, :]  # [4, T]
        # bj[h, p, jt] = slope_h * (jt*128 + p) / 8
        bj = hs[:, None, None] * (jt_idx[None, None, :] * 128 + p_idx[None, :, None]) / 8.0
        bias = (
            np.broadcast_to(b_proj, (128, D)).copy()
            if g == 0
            else np.zeros((128, D), dtype=np.float32)
        )
        in_maps.append(
            {
                "x": np.ascontiguousarray(x[b]),
                "wq": np.ascontiguousarray(W_query[:, g * 256 : (g + 1) * 256]),
                "wk": np.ascontiguousarray(W_key[:, g * 256 : (g + 1) * 256]),
                "wv": np.ascontiguousarray(W_value[:, g * 256 : (g + 1) * 256]),
                "wp": np.ascontiguousarray(W_proj[g * 256 : (g + 1) * 256, :]),
                "qaug": qaug.astype(np.float32),
                "kones": kones,
                "vones": vones,
                "iden": iden,
                "bj": bj.astype(np.float32),
                "bias": bias.astype(np.float32),
            }
        )
    return in_maps


def assemble(results):
    out = np.empty((B, T, D), dtype=np.float32)
    for b in range(B):
        acc = results[b * HL]["out"].astype(np.float32).copy()
        for g in range(1, HL):
            acc += results[b * HL + g]["out"]
        out[b] = acc
    return out


def kernel(x, W_query, W_key, W_value, W_proj, b_proj, **run_kwargs):
    nc = get_nc()
    in_maps = make_in_maps(x, W_query, W_key, W_value, W_proj, b_proj)
    res = run_bass_kernel_spmd(nc, in_maps, core_ids=list(range(NCORES)), **run_kwargs)
    out = assemble(res.results)
    kernel.last_result = res
    return out


# revision 21
# speedup vs baseline: 1.1680x; 1.1680x over previous
"""Trainium2 Bass kernel for causal multi-head attention with ALiBi.

Module: y = proj(softmax((causal_mask(q k^T) + alibi)/sqrt(dh)) v) + b
Shapes: x [2, 2048, 1024], H=16 heads, dh=64, fp32.

Sharding over 8 cores: core c -> batch b = c//4, head group g = c%4
(heads 4g..4g+3). Each core computes QKV for its 4 heads, causal ALiBi
attention, and its row-slice of the output projection; the host sums the
4 row-parallel partials per batch (b_proj is added by the g==0 cores).

Device-side structure:
- All matmuls float32r (full PE rate; plain fp32 runs at 1/4).
- q/k head-transposed [65, 2048]: rows 0:64 head dims, row 64 is an
  augmentation row (k: ones, q: -slope*i) so the scores matmul yields
  qk - slope*i; the per-column term cancels in softmax and only bounds
  the exponents. The +slope*j ALiBi half enters exactly through the ACT
  per-partition bias. Scores are computed transposed [j, i].
- exp with no max pass; causal handled by skipping fully-masked j-tiles
  and one wide affine_select zero-fill over the 4 diagonal j-tiles per
  (i-slice, head), which are exp'd into one contiguous [128, 2048] tile.
- ctx matmul lhsT = [64 v-cols | 64 ones-cols]: psum rows 0:64 get the
  unnormalized context, rows 64:128 get 64 replicated copies of the
  softmax denominator -> reciprocal + one DVE multiply normalizes.
- Phase A/B fused per 512-row group: x DMA -> PE transpose -> q/k matmul
  for that i-slice -> v matmul for those j-tiles, so DMA and compute
  overlap from the start.
- Phase C: two heads' streams interleave, and each block's tail work
  (diagonal ctx, reciprocal, normalize, projection) drains into the next
  block's scores stream so the PE never stalls on the mask/normalize
  chain.
"""

import numpy as np

import concourse.bass as bass
import concourse.mybir as mybir
import concourse.tile as tile
from concourse import bacc
from concourse.bass_utils import run_bass_kernel_spmd

F32 = mybir.dt.float32
F32R = mybir.dt.float32r
AF = mybir.ActivationFunctionType

B, T, D = 2, 2048, 1024
H = 16
DH = 64
HL = 4            # heads per core
NCORES = 8
NT = T // 128     # 16 j-tiles
NS = T // 512     # 4 i-slices
KC = D // 128     # 8 contraction chunks over D
CTX_LAG = 3       # ctx matmuls trail their exp by this many emission steps
DRAIN_PER_STEP = 2


def build_nc():
    nc = bacc.Bacc("TRN2", target_bir_lowering=False, debug=False)

    x_d = nc.dram_tensor("x", [T, D], F32R, kind="ExternalInput")
    wq_d = nc.dram_tensor("wq", [D, HL * DH], F32R, kind="ExternalInput")
    wk_d = nc.dram_tensor("wk", [D, HL * DH], F32R, kind="ExternalInput")
    wv_d = nc.dram_tensor("wv", [D, HL * DH], F32R, kind="ExternalInput")
    wp_d = nc.dram_tensor("wp", [HL * DH, D], F32R, kind="ExternalInput")
    qaug_d = nc.dram_tensor("qaug", [HL, T], F32R, kind="ExternalInput")
    kones_d = nc.dram_tensor("kones", [1, T], F32R, kind="ExternalInput")
    vones_d = nc.dram_tensor("vones", [128, 64], F32R, kind="ExternalInput")
    iden_d = nc.dram_tensor("iden", [128, 128], F32R, kind="ExternalInput")
    bj_d = nc.dram_tensor("bj", [HL, 128, NT], F32, kind="ExternalInput")
    bias_d = nc.dram_tensor("bias", [128, D], F32, kind="ExternalInput")
    out_d = nc.dram_tensor("out", [T, D], F32, kind="ExternalOutput")

    with tile.TileContext(nc) as tc:
        with tc.tile_pool(name="persist", bufs=1) as pp:
            qTa = [pp.tile([65, T], F32R, name=f"qTa{h}") for h in range(HL)]
            kTa = [pp.tile([65, T], F32R, name=f"kTa{h}") for h in range(HL)]
            vA = [pp.tile([128, 384], F32R, name=f"vA{j}") for j in range(NT)]

            hwq = [nc.sync, nc.scalar]

            def load_constants():
                for h in range(HL):
                    nc.gpsimd.dma_start(qTa[h][64:65, :], qaug_d[h : h + 1, :])
                    nc.gpsimd.dma_start(kTa[h][64:65, :], kones_d[:, :])
                for j in range(NT):
                    nc.gpsimd.dma_start(vA[j][:, 64:128], vones_d[:])
                    nc.gpsimd.dma_start(vA[j][:, 256:320], vones_d[:])

            # ---- Phase A+B fused: per row-group transpose + QKV ----
            with (
                tc.tile_pool(name="ab", bufs=1) as ab,
                tc.tile_pool(name="abr", bufs=7) as abr,
                tc.tile_pool(name="psab", bufs=2, space="PSUM") as psab,
            ):
                idn = ab.tile([128, 128], F32R, name="idn")
                nc.scalar.dma_start(idn[:], iden_d[:])
                xT = [ab.tile([128, T], F32R, name=f"xT{d}") for d in range(KC)]
                wqs = [ab.tile([128, HL * DH], F32R, name=f"wqs{d}") for d in range(KC)]
                wks = [ab.tile([128, HL * DH], F32R, name=f"wks{d}") for d in range(KC)]
                wvs = [ab.tile([128, HL * DH], F32R, name=f"wvs{d}") for d in range(KC)]

                def load_weights():
                    n = 0
                    for d in range(KC):
                        hwq[n % 2].dma_start(
                            wqs[d][:], wq_d[d * 128 : (d + 1) * 128, :]
                        )
                        hwq[(n + 1) % 2].dma_start(
                            wks[d][:], wk_d[d * 128 : (d + 1) * 128, :]
                        )
                        hwq[n % 2].dma_start(
                            wvs[d][:], wv_d[d * 128 : (d + 1) * 128, :]
                        )
                        n += 1

                for sg in range(NS):
                    # 1. DMA this row-group's x tiles (alternating HW queues)
                    xts = []
                    for i4 in range(4):
                        it = sg * 4 + i4
                        xt = abr.tile([128, D], F32R, name="xt", tag="xt")
                        hwq[i4 % 2].dma_start(
                            xt[:], x_d[it * 128 : (it + 1) * 128, :]
                        )
                        xts.append(xt)
                    if sg == 0:
                        load_weights()
                        load_constants()

                    # 2. transpose into xT[:, sg-slice] via PE matmul with I
                    for d in range(KC):
                        pt = psab.tile([128, 512], F32, name="pt", tag="pt")
                        for i4 in range(4):
                            nc.tensor.matmul(
                                pt[:, i4 * 128 : (i4 + 1) * 128],
                                xts[i4][:, d * 128 : (d + 1) * 128],
                                idn[:],
                                start=True,
                                stop=True,
                            )
                        nc.vector.tensor_copy(
                            xT[d][:, sg * 512 : (sg + 1) * 512], pt[:]
                        )

                    # 3. q/k projections for i-slice sg
                    for ws, dst in ((wqs, qTa), (wks, kTa)):
                        for p2 in range(2):
                            pq = psab.tile([128, 512], F32, name="pq", tag="pq")
                            for d in range(KC):
                                nc.tensor.matmul(
                                    pq[:],
                                    ws[d][:, p2 * 128 : (p2 + 1) * 128],
                                    xT[d][:, sg * 512 : (sg + 1) * 512],
                                    start=(d == 0),
                                    stop=(d == KC - 1),
                                )
                            nc.vector.tensor_copy(
                                dst[2 * p2][0:64, sg * 512 : (sg + 1) * 512],
                                pq[0:64, :],
                            )
                            nc.vector.tensor_copy(
                                dst[2 * p2 + 1][0:64, sg * 512 : (sg + 1) * 512],
                                pq[64:128, :],
                            )

                    # 4. v projection for this group's j-tiles
                    for i4 in range(4):
                        j = sg * 4 + i4
                        pv = psab.tile([128, HL * DH], F32, name="pv", tag="pv")
                        for d in range(KC):
                            nc.tensor.matmul(
                                pv[:],
                                xT[d][:, j * 128 : (j + 1) * 128],
                                wvs[d][:],
                                start=(d == 0),
                                stop=(d == KC - 1),
                            )
                        nc.vector.tensor_copy(vA[j][:, 0:64], pv[:, 0:64])
                        nc.vector.tensor_copy(vA[j][:, 128:256], pv[:, 64:192])
                        nc.vector.tensor_copy(vA[j][:, 320:384], pv[:, 192:256])

            # ---- Phase C+D: attention + projection, cross-block pipelined ----
            with (
                tc.tile_pool(name="attn", bufs=1) as at,
                tc.tile_pool(name="attnr", bufs=4) as atr,
                tc.tile_pool(name="pssc", bufs=3, space="PSUM") as pssc,
                tc.tile_pool(name="pscx", bufs=3, space="PSUM") as pscx,
                tc.tile_pool(name="psot", bufs=2, space="PSUM") as psot,
            ):
                wps = [at.tile([128, D], F32R, name=f"wps{t}") for t in range(2)]
                bias_sb = at.tile([128, D], F32, name="bias_sb")
                ctxT = [at.tile([128, T], F32R, name=f"ctxT{t}") for t in range(2)]
                bjs = [at.tile([128, NT], F32, name=f"bjs{h}") for h in range(HL)]
                for t in range(2):
                    nc.sync.dma_start(wps[t][:], wp_d[t * 128 : (t + 1) * 128, :])
                nc.sync.dma_start(bias_sb[:], bias_d[:])
                for h in range(HL):
                    nc.sync.dma_start(bjs[h][:], bj_d[h])

                pending = []  # deferred tail closures, drained into next block
                proj_queue = []  # proj units wait one extra block

                def drain(k):
                    for _ in range(k):
                        if pending:
                            pending.pop(0)()

                def emit_head_pair(s, heads, promote_projs=False):
                    """Interleave two heads' scores/exp/ctx; defer the diag-ctx
                    + normalize tail into the next block via `pending`."""
                    if promote_projs:
                        pending.extend(proj_queue)
                        proj_queue.clear()
                    nj = 4 * (s + 1)
                    diag = list(range(4 * s, 4 * s + 4))
                    off = list(range(0, 4 * s))
                    jseq = diag + off          # scores/exp emission order
                    cons = off + diag          # ctx emission order
                    st = {
                        h: dict(
                            pcx=pscx.tile([128, 512], F32, name="pcx", tag="pcx"),
                            exd=atr.tile(
                                [128, 2048], F32R, name="exd", tag="exd", bufs=4
                            ),
                            exs={},
                            exstep={},
                            ci=0,
                        )
                        for h in heads
                    }

                    def scores_exp(h, idx, step):
                        j = jseq[idx]
                        i = st[h]
                        psc = pssc.tile(
                            [128, 512], F32, name="psc", tag="psc", bufs=3
                        )
                        nc.tensor.matmul(
                            psc[:],
                            kTa[h][:, j * 128 : (j + 1) * 128],
                            qTa[h][:, s * 512 : (s + 1) * 512],
                            start=True,
                            stop=True,
                        )
                        if idx < 4:
                            dst = i["exd"][:, idx * 512 : (idx + 1) * 512]
                        else:
                            dst = atr.tile(
                                [128, 512], F32R, name="ex", tag="ex", bufs=6
                            )[:]
                        nc.scalar.activation(
                            dst, psc[:], AF.Exp,
                            bias=bjs[h][:, j : j + 1], scale=0.125,
                        )
                        i["exs"][j] = dst
                        i["exstep"][j] = step
                        if idx == 3:
                            # one mask over all 4 diagonal tiles:
                            # keep where f - 128*blk - p >= 0
                            nc.gpsimd.affine_select(
                                out=i["exd"][:], in_=i["exd"][:],
                                compare_op=mybir.AluOpType.is_ge,
                                fill=0.0, base=0,
                                pattern=[[-128, 4], [1, 512]],
                                channel_multiplier=-1,
                            )

                    VA_OFF = [0, 64, 192, 256]

                    def ctx_one(h, ci):
                        i = st[h]
                        j = cons[ci]
                        o = VA_OFF[h]
                        nc.tensor.matmul(
                            i["pcx"][:],
                            vA[j][:, o : o + 128],
                            i["exs"][j][:],
                            start=(ci == 0),
                            stop=(ci == nj - 1),
                        )

                    def try_offdiag_ctx(step_now):
                        for h in heads:
                            i = st[h]
                            while i["ci"] < len(off):  # off-diag only
                                j = cons[i["ci"]]
                                if j not in i["exs"]:
                                    break
                                if step_now < i["exstep"][j] + CTX_LAG:
                                    break
                                ctx_one(h, i["ci"])
                                i["ci"] += 1

                    step = 0
                    for idx in range(nj):
                        for h in heads:
                            scores_exp(h, idx, step)
                            step += 1
                            drain(DRAIN_PER_STEP)
                            try_offdiag_ctx(step)

                    # tail: remaining off-diag + diag ctx, then normalize
                    def make_tail(h):
                        i = st[h]

                        def fin_ctx(ci):
                            return lambda: ctx_one(h, ci)

                        items = [fin_ctx(ci) for ci in range(i["ci"], nj)]
                        i["ci"] = nj

                        def norm():
                            rs = atr.tile([64, 512], F32, name="rs", tag="rs")
                            ctx_rows = (0, 64) if h % 2 == 0 else (64, 128)
                            rs_rows = (64, 128) if h % 2 == 0 else (0, 64)
                            nc.vector.reciprocal(
                                rs[:], i["pcx"][rs_rows[0] : rs_rows[1], :]
                            )
                            nc.vector.tensor_mul(
                                ctxT[h // 2][
                                    (h % 2) * 64 : (h % 2) * 64 + 64,
                                    s * 512 : (s + 1) * 512,
                                ],
                                i["pcx"][ctx_rows[0] : ctx_rows[1], :],
                                rs[:],
                            )

                        items.append(norm)
                        return items

                    for h in heads:
                        pending.extend(make_tail(h))

                def proj_unit(s, i4, n2):
                    def run():
                        it = s * 4 + i4
                        po = psot.tile([128, 512], F32, name="po", tag="po")
                        for t in range(2):
                            nc.tensor.matmul(
                                po[:],
                                ctxT[t][:, it * 128 : (it + 1) * 128],
                                wps[t][:, n2 * 512 : (n2 + 1) * 512],
                                start=(t == 0),
                                stop=(t == 1),
                            )
                        ot = atr.tile([128, 512], F32, name="ot", tag="ot")
                        nc.vector.tensor_add(
                            ot[:], po[:], bias_sb[:, n2 * 512 : (n2 + 1) * 512]
                        )
                        nc.sync.dma_start(
                            out_d[
                                it * 128 : (it + 1) * 128,
                                n2 * 512 : (n2 + 1) * 512,
                            ],
                            ot[:],
                        )

                    return run

                for s in range(NS):
                    emit_head_pair(s, (0, 1))
                    emit_head_pair(s, (2, 3), promote_projs=True)
                    for i4 in range(4):
                        for n2 in range(2):
                            proj_queue.append(proj_unit(s, i4, n2))
                drain(len(pending))
                pending.extend(proj_queue)
                proj_queue.clear()
                drain(len(pending))
    nc.compile()
    return nc


_NC = None


def get_nc():
    global _NC
    if _NC is None:
        _NC = build_nc()
    return _NC


def make_in_maps(x, W_query, W_key, W_value, W_proj, b_proj):
    x = np.asarray(x, dtype=np.float32)
    W_query = np.asarray(W_query, dtype=np.float32)
    W_key = np.asarray(W_key, dtype=np.float32)
    W_value = np.asarray(W_value, dtype=np.float32)
    W_proj = np.asarray(W_proj, dtype=np.float32)
    b_proj = np.asarray(b_proj, dtype=np.float32)

    slopes = 2.0 ** (-8.0 * np.arange(1, H + 1, dtype=np.float32) / H)
    i_idx = np.arange(T, dtype=np.float32)
    p_idx = np.arange(128, dtype=np.float32)
    jt_idx = np.arange(NT, dtype=np.float32)
    iden = np.eye(128, dtype=np.float32)
    vones = np.ones((128, 64), dtype=np.float32)
    kones = np.ones((1, T), dtype=np.float32)

    in_maps = []
    for c in range(NCORES):
        b, g = divmod(c, HL)
        hs = slopes[g * HL : (g + 1) * HL]  # [4]
        qaug = -hs[:, None] * i_idx[None, :]  # [4, T]
        # bj[h, p, jt] = slope_h * (jt*128 + p) / 8
        bj = hs[:, None, None] * (jt_idx[None, None, :] * 128 + p_idx[None, :, None]) / 8.0
        bias = (
            np.broadcast_to(b_proj, (128, D)).copy()
            if g == 0
            else np.zeros((128, D), dtype=np.float32)
        )
        in_maps.append(
            {
                "x": np.ascontiguousarray(x[b]),
                "wq": np.ascontiguousarray(W_query[:, g * 256 : (g + 1) * 256]),
                "wk": np.ascontiguousarray(W_key[:, g * 256 : (g + 1) * 256]),
                "wv": np.ascontiguousarray(W_value[:, g * 256 : (g + 1) * 256]),
                "wp": np.ascontiguousarray(W_proj[g * 256 : (g + 1) * 256, :]),
                "qaug": qaug.astype(np.float32),
                "kones": kones,
                "vones": vones,
                "iden": iden,
                "bj": bj.astype(np.float32),
                "bias": bias.astype(np.float32),
            }
        )
    return in_maps


def assemble(results):
    out = np.empty((B, T, D), dtype=np.float32)
    for b in range(B):
        acc = results[b * HL]["out"].astype(np.float32).copy()
        for g in range(1, HL):
            acc += results[b * HL + g]["out"]
        out[b] = acc
    return out


def kernel(x, W_query, W_key, W_value, W_proj, b_proj, **run_kwargs):
    nc = get_nc()
    in_maps = make_in_maps(x, W_query, W_key, W_value, W_proj, b_proj)
    res = run_bass_kernel_spmd(nc, in_maps, core_ids=list(range(NCORES)), **run_kwargs)
    out = assemble(res.results)
    kernel.last_result = res
    return out
